# revision 16
# baseline (speedup 1.0000x reference)
"""AdderNet 2D convolution (negative L1 distance conv) on 8 TRN2 NeuronCores.

Problem: x [4,64,64,32] f32, kernel [3,3,32,32] f32 ->
    out[n,h,w,c] = -sum_{dy,dx,ci} |x[n,h+dy-1,w+dx-1,ci] - kernel[dy,dx,ci,c]|
(SAME zero padding, stride 1), out [4,64,64,32] f32.

Algorithm: |u| = u + 2*relu(-u), so with d = (dy,dx,ci):
    out[c,m] = -sum_d x[m,d] + sum_d W[d,c] - 2*sum_d relu(W[d,c] - x[m,d])
  * sum_d x needs NO elementwise work: TensorE ones-reduce over x directly.
  * sum_d W[d,c] is a host constant, folded into the output-copy bias.
  * relu(W - x) is ONE dual-op instruction per pass on VectorE
    (tensor_scalar(add, max) in 4x bf16 mode), ScalarE (activation Relu,
    per-partition bias) or GpSimdE.

Distribution (data-parallel over output rows, no collectives):
  - Each of the 8 cores owns 32 output rows (half of one image).
  - Host pre-builds, per core, three "tap-group" tensors xn[g] [128, 2112]
    bf16 holding NEGATED shifted copies of the core's input slab:
    partitions = 4 blocks x 32 input channels (block b of group g = tap
    t=4g+b; g=2: tap 8 replicated), free axis = 32 rows x 66 padded cols.
  - 72 relu passes (one per (tap-group, out-channel set)); TensorE reduces
    partitions with a (-2)-one-hot matmul accumulated into PSUM, 4 column
    stripes of the PE array running concurrently via tile_position.
  - PSUM -> SBUF (+S_w bias) -> DMA out; host unscrambles to NHWC f32.
"""
import numpy as np
import ml_dtypes

H, W, CIN, COUT = 64, 64, 32, 32
ROWS = 32            # output rows per core
WP = 66              # padded row width (64 + 2)
F = ROWS * WP        # 2112 free-axis size per core
N_CORES = 8
CHUNKS = [(0, 512), (512, 512), (1024, 512), (1536, 512), (2048, 64)]
N_ACT = 22           # relu passes on ScalarE
N_GP = 0             # relu passes on GpSimdE

_BF16 = ml_dtypes.bfloat16


# ----------------------------------------------------------------- host prep
def _host_prep_core(x, core):
    """xn [3, 128, F] f32 (negated shifted slabs) for one core."""
    n, h0 = core // 2, (core % 2) * 32
    xs = np.zeros((34, WP, CIN), np.float32)
    lo, hi = max(0, h0 - 1), min(H, h0 + 33)
    xs[lo - (h0 - 1): hi - (h0 - 1), 1:65] = x[n, lo:hi]
    XT = np.zeros((3, 128, F), np.float32)
    for g in range(3):
        for b in range(4):
            t = 4 * g + b if g < 2 else 8
            dy, dx = divmod(t, 3)
            sh = np.zeros((ROWS, WP, CIN), np.float32)
            qlo, qhi = max(0, 1 - dx), min(WP, WP + 1 - dx)
            sh[:, qlo:qhi] = xs[dy:dy + ROWS, qlo + dx - 1: qhi + dx - 1]
            XT[g, 32 * b:32 * b + 32] = sh.transpose(2, 0, 1).reshape(CIN, F)
    return -XT


def _build_passes():
    """sx passes (PE-only sum_d x reduction) + 72 relu passes.

    relu pass modes:
      A (g=0,1): one channel c across 4 tap blocks; lhsT pattern pat=c%8
          (col c%8 = -2 on all 128 partitions).
      B (g=2): tap 8 replicated; channels c=4k..4k+3 block-diagonal;
          lhsT pattern 8+(k%2) (-2 entries).
    sx passes use pattern 10 (+1 everywhere, since xn = -x).
    """
    sx = []
    for g in range(3):
        for j in range(4):
            sx.append(dict(kind="sx", g=g, stripe=j, pat=10 if g < 2 else 11,
                           start=(g == 0)))
    passes = []
    c_order = [8 * j + i for i in range(8) for j in range(4)]
    for g in (0, 1):
        for c in c_order:
            passes.append(dict(kind="r", mode="A", g=g, c=c,
                               stripe=c // 8, pat=c % 8))
    for i in range(2):
        for j in range(4):
            k = 2 * j + i
            passes.append(dict(kind="r", mode="B", g=2, k=k,
                               stripe=k // 2, pat=8 + (k % 2)))
    last = {}
    for idx, p in enumerate(passes):
        p["start"] = False
        p["stop"] = False
        last[p["stripe"]] = idx
    for idx in last.values():
        passes[idx]["stop"] = True
    # 3-way engine split, ScalarE/GpSimd spread evenly
    n = len(passes)
    for i, p in enumerate(passes):
        if (i * N_ACT) // n != ((i + 1) * N_ACT) // n:
            p["engine"] = "a"
        elif ((i + 1) * N_GP) // n != ((i + 2) * N_GP) // n:
            p["engine"] = "g"
        else:
            p["engine"] = "v"
    return sx, passes


def _host_prep_weights(kf):
    """wp [128, 72] f32 (+W cols per pass), lt [128, 96] bf16, sw [128,1] f32."""
    W_col = kf.reshape(-1, COUT)  # [288, 32], d = (tap, ci)
    sx, passes = _build_passes()
    wp = np.zeros((128, 72), np.float32)
    for i, p in enumerate(passes):
        if p["mode"] == "A":
            g, c = p["g"], p["c"]
            for b in range(4):
                d = (4 * g + b) * 32
                wp[32 * b:32 * b + 32, i] = W_col[d:d + 32, c]
        else:
            k = p["k"]
            for b in range(4):
                wp[32 * b:32 * b + 32, i] = W_col[8 * 32:8 * 32 + 32, 4 * k + b]
    lt = np.zeros((128, 96), np.float32)
    for r in range(8):                      # patterns 0..7: col r = -2 everywhere
        lt[:, 8 * r + r] = -2.0
    for s in (0, 1):                        # patterns 8,9: block diagonal -2
        for b in range(4):
            lt[32 * b:32 * b + 32, 8 * (8 + s) + 4 * s + b] = -2.0
    lt[:, 80:88] = 1.0                      # pattern 10: all ones (sum_d x)
    lt[:32, 88:96] = 1.0                    # pattern 11: block-0 ones (g=2 sx;
                                            # tap 8 is replicated in 4 blocks)
    sw = np.zeros((128, 1), np.float32)
    s_w = W_col.sum(axis=0)                 # [32]
    for c in range(COUT):
        sw[32 * (c // 8) + (c % 8), 0] = s_w[c]
    return wp, lt.astype(_BF16), sw


# ------------------------------------------------------------- device kernel
def _build_nc():
    from contextlib import ExitStack
    import concourse.tile as tile
    from concourse import bacc, mybir

    bf16, f32 = mybir.dt.bfloat16, mybir.dt.float32
    Alu = mybir.AluOpType
    Act = mybir.ActivationFunctionType

    sx_passes, passes = _build_passes()
    nc = bacc.Bacc("TRN2", target_bir_lowering=False, debug=False)
    xn_d = [nc.declare_dram_parameter(f"xn{g}", [128, F], bf16, False)
            for g in range(3)]
    wp_d = nc.declare_dram_parameter("wp", [128, 72], f32, False)
    lt_d = nc.declare_dram_parameter("lt", [128, 96], bf16, False)
    sw_d = nc.declare_dram_parameter("sw", [128, 1], f32, False)
    o_d = nc.declare_dram_parameter("o", [4, 8, F], bf16, True)

    with tile.TileContext(nc) as tc, ExitStack() as ctx:
        singles = ctx.enter_context(tc.tile_pool(name="singles", bufs=1))
        bvpool = ctx.enter_context(tc.tile_pool(name="bvpool", bufs=8))
        bapool = ctx.enter_context(tc.tile_pool(name="bapool", bufs=4))
        ppool = ctx.enter_context(tc.tile_pool(name="ppool", bufs=1, space="PSUM"))

        wp = singles.tile([128, 72], f32, tag="wp")
        lt = singles.tile([128, 96], bf16, tag="lt")
        sw = singles.tile([128, 1], f32, tag="sw")
        nc.scalar.dma_start(lt[:], lt_d[:])
        nc.scalar.dma_start(wp[:], wp_d[:])
        nc.scalar.dma_start(sw[:], sw_d[:])
        xn = []
        for g in range(3):
            t = singles.tile([128, F], bf16, tag=f"xn{g}")
            xn.append(t)
        nc.sync.dma_start(xn[0][:], xn_d[0][:])
        nc.gpsimd.dma_start(xn[1][:], xn_d[1][:])
        nc.sync.dma_start(xn[2][:], xn_d[2][:])
        ost = singles.tile([128, F], bf16, tag="ost")
        P = ppool.tile([128, F], f32, tag="P")

        # Touchers: pre-sync each compute engine on the input DMAs so most
        # compute ops need a single semaphore wait.
        tv = singles.tile([128, 8], f32, tag="tv")
        ta = singles.tile([128, 8], f32, tag="ta")
        tg = singles.tile([128, 8], f32, tag="tg")
        nc.vector.tensor_copy(tv[:, 0:2], wp[:, 0:2])
        nc.scalar.copy(ta[:, 0:2], wp[:, 0:2])
        nc.gpsimd.tensor_copy(tg[:, 0:2], wp[:, 0:2])
        for g in range(3):
            nc.vector.tensor_copy(tv[:, 2 * g + 2:2 * g + 4], xn[g][:, 0:2])
            nc.scalar.copy(ta[:, 2 * g + 2:2 * g + 4], xn[g][:, 0:2])
            nc.gpsimd.tensor_copy(tg[:, 2 * g + 2:2 * g + 4], xn[g][:, 0:2])
        nc.tensor.ldweights(lt[:, 80:88])

        def emit_mms(rhs_tile, p):
            j = p["stripe"]
            lt_ap = lt[:, 8 * p["pat"]:8 * p["pat"] + 8]
            for (off, sz) in CHUNKS:
                nc.tensor.matmul(
                    P[32 * j:32 * j + 8, off:off + sz],
                    lt_ap, rhs_tile[:, off:off + sz],
                    start=p["start"], stop=p.get("stop", False),
                    tile_position=(0, 32 * j),
                )

        for p in sx_passes:
            emit_mms(xn[p["g"]], p)

        for i, p in enumerate(passes):
            scol = wp[:, i:i + 1]
            src = xn[p["g"]]
            if p["engine"] == "v":
                B = bvpool.tile([128, F], bf16, tag="BV")
                nc.vector.tensor_scalar(B[:], src[:], scol, 0.0,
                                        op0=Alu.add, op1=Alu.max)
            elif p["engine"] == "g":
                B = bapool.tile([128, F], bf16, tag="BG")
                nc.gpsimd.tensor_scalar(B[:], src[:], scol, 0.0,
                                        op0=Alu.add, op1=Alu.max)
            else:
                B = bapool.tile([128, F], bf16, tag="BA")
                nc.scalar.activation(B[:], src[:], Act.Relu, bias=scol)
            emit_mms(B, p)

        # epilogue: out = psum + S_w[c] (bias per partition).  One whole-psum
        # copy: engine cost scales with free size only, and a single
        # instruction avoids PSUM bank-overlap serialization.
        # split at a PSUM bank boundary (1024 f32) so the two copies touch
        # disjoint banks and can run concurrently on VectorE/ScalarE.
        nc.vector.tensor_scalar(ost[:, 0:1024], P[:, 0:1024], sw[:], None,
                                op0=Alu.add)
        nc.scalar.activation(ost[:, 1024:F], P[:, 1024:F], Act.Identity,
                             bias=sw[:])
        for j in range(4):
            nc.sync.dma_start(o_d[j], ost[32 * j:32 * j + 8, :])
    nc.finalize()
    return nc


_NC_CACHE = None


def _get_nc():
    global _NC_CACHE
    if _NC_CACHE is None:
        _NC_CACHE = _build_nc()
    return _NC_CACHE


# -------------------------------------------------------------------- driver
def _run(x, kf, trace=False):
    from concourse.bass_utils import run_bass_kernel_spmd

    x = np.ascontiguousarray(np.asarray(x, np.float32))
    kf = np.ascontiguousarray(np.asarray(kf, np.float32))
    wp, lt, sw = _host_prep_weights(kf)
    in_maps = []
    for core in range(N_CORES):
        XN = _host_prep_core(x, core)
        in_maps.append({
            "xn0": XN[0].astype(_BF16),
            "xn1": XN[1].astype(_BF16),
            "xn2": XN[2].astype(_BF16),
            "wp": wp,
            "lt": lt,
            "sw": sw,
        })
    nc = _get_nc()
    res = run_bass_kernel_spmd(nc, in_maps, core_ids=list(range(N_CORES)),
                               trace=trace)
    out = np.zeros((4, H, W, COUT), np.float32)
    for core in range(N_CORES):
        o = np.asarray(res.results[core]["o"]).astype(np.float32)  # [4, 8, F]
        n, h0 = core // 2, (core % 2) * 32
        oo = o.reshape(4, 8, ROWS, WP)[:, :, :, 1:65]   # [4, 8, 32, 64]
        out[n, h0:h0 + 32] = oo.transpose(2, 3, 0, 1).reshape(ROWS, W, COUT)
    return out, res


def kernel(**inputs):
    out, _ = _run(inputs["x"], inputs["kernel"])
    return out


# revision 17
# speedup vs baseline: 1.0611x; 1.0611x over previous
"""AdderNet 2D convolution (negative L1 distance conv) on 8 TRN2 NeuronCores.

Problem: x [4,64,64,32] f32, kernel [3,3,32,32] f32 ->
    out[n,h,w,c] = -sum_{dy,dx,ci} |x[n,h+dy-1,w+dx-1,ci] - kernel[dy,dx,ci,c]|
(SAME zero padding, stride 1), out [4,64,64,32] f32.

Algorithm: |u| = u + 2*relu(-u), so with d = (dy,dx,ci):
    out[c,m] = -sum_d x[m,d] + sum_d W[d,c] - 2*sum_d relu(W[d,c] - x[m,d])
  * sum_d x needs NO elementwise work: TensorE ones-reduce over x directly.
  * sum_d W[d,c] is a host constant, folded into the output-copy bias.
  * relu(W - x) is ONE dual-op instruction per pass on VectorE
    (tensor_scalar(add, max) in 4x bf16 mode), ScalarE (activation Relu,
    per-partition bias) or GpSimdE.

Distribution (data-parallel over output rows, no collectives):
  - Each of the 8 cores owns 32 output rows (half of one image).
  - Host pre-builds, per core, three "tap-group" tensors xn[g] [128, 2112]
    bf16 holding NEGATED shifted copies of the core's input slab:
    partitions = 4 blocks x 32 input channels (block b of group g = tap
    t=4g+b; g=2: tap 8 replicated), free axis = 32 rows x 66 padded cols.
  - 72 relu passes (one per (tap-group, out-channel set)); TensorE reduces
    partitions with a (-2)-one-hot matmul accumulated into PSUM, 4 column
    stripes of the PE array running concurrently via tile_position.
  - PSUM -> SBUF (+S_w bias) -> DMA out; host unscrambles to NHWC f32.
"""
import numpy as np
import ml_dtypes

H, W, CIN, COUT = 64, 64, 32, 32
ROWS = 32            # output rows per core
WP = 66              # padded row width (64 + 2)
F = ROWS * WP        # 2112 free-axis size per core
N_CORES = 8
CHUNKS = [(0, 512), (512, 512), (1024, 512), (1536, 512), (2048, 64)]
N_ACT = 20           # relu passes on ScalarE
N_GP = 0             # relu passes on GpSimdE

_BF16 = ml_dtypes.bfloat16


# ----------------------------------------------------------------- host prep
def _host_prep_core(x, core):
    """xn [3, 128, F] f32 (negated shifted slabs) for one core."""
    n, h0 = core // 2, (core % 2) * 32
    xs = np.zeros((34, WP, CIN), np.float32)
    lo, hi = max(0, h0 - 1), min(H, h0 + 33)
    xs[lo - (h0 - 1): hi - (h0 - 1), 1:65] = x[n, lo:hi]
    XT = np.zeros((3, 128, F), np.float32)
    for g in range(3):
        for b in range(4):
            t = 4 * g + b if g < 2 else 8
            dy, dx = divmod(t, 3)
            sh = np.zeros((ROWS, WP, CIN), np.float32)
            qlo, qhi = max(0, 1 - dx), min(WP, WP + 1 - dx)
            sh[:, qlo:qhi] = xs[dy:dy + ROWS, qlo + dx - 1: qhi + dx - 1]
            XT[g, 32 * b:32 * b + 32] = sh.transpose(2, 0, 1).reshape(CIN, F)
    return -XT


def _build_passes():
    """sx passes (PE-only sum_d x reduction) + 72 relu passes.

    relu pass modes:
      A (g=0,1): one channel c across 4 tap blocks; lhsT pattern pat=c%8
          (col c%8 = -2 on all 128 partitions).
      B (g=2): tap 8 replicated; channels c=4k..4k+3 block-diagonal;
          lhsT pattern 8+(k%2) (-2 entries).
    sx passes use pattern 10 (+1 everywhere, since xn = -x).
    """
    sx = []
    for g in range(3):
        for j in range(4):
            sx.append(dict(kind="sx", g=g, stripe=j, pat=10 if g < 2 else 11,
                           start=(g == 0)))
    passes = []
    c_order = [8 * j + i for i in range(8) for j in range(4)]
    for g in (0, 1):
        for c in c_order:
            passes.append(dict(kind="r", mode="A", g=g, c=c,
                               stripe=c // 8, pat=c % 8))
    for i in range(2):
        for j in range(4):
            k = 2 * j + i
            passes.append(dict(kind="r", mode="B", g=2, k=k,
                               stripe=k // 2, pat=8 + (k % 2)))
    last = {}
    for idx, p in enumerate(passes):
        p["start"] = False
        p["stop"] = False
        last[p["stripe"]] = idx
    for idx in last.values():
        passes[idx]["stop"] = True
    # 3-way engine split, ScalarE/GpSimd spread evenly
    n = len(passes)
    for i, p in enumerate(passes):
        if (i * N_ACT) // n != ((i + 1) * N_ACT) // n:
            p["engine"] = "a"
        elif ((i + 1) * N_GP) // n != ((i + 2) * N_GP) // n:
            p["engine"] = "g"
        else:
            p["engine"] = "v"
    return sx, passes


def _host_prep_weights(kf):
    """wp [128, 72] f32 (+W cols per pass), lt [128, 96] bf16, sw [128,1] f32."""
    W_col = kf.reshape(-1, COUT)  # [288, 32], d = (tap, ci)
    sx, passes = _build_passes()
    wp = np.zeros((128, 72), np.float32)
    for i, p in enumerate(passes):
        if p["mode"] == "A":
            g, c = p["g"], p["c"]
            for b in range(4):
                d = (4 * g + b) * 32
                wp[32 * b:32 * b + 32, i] = W_col[d:d + 32, c]
        else:
            k = p["k"]
            for b in range(4):
                wp[32 * b:32 * b + 32, i] = W_col[8 * 32:8 * 32 + 32, 4 * k + b]
    lt = np.zeros((128, 96), np.float32)
    for r in range(8):                      # patterns 0..7: col r = -2 everywhere
        lt[:, 8 * r + r] = -2.0
    for s in (0, 1):                        # patterns 8,9: block diagonal -2
        for b in range(4):
            lt[32 * b:32 * b + 32, 8 * (8 + s) + 4 * s + b] = -2.0
    lt[:, 80:88] = 1.0                      # pattern 10: all ones (sum_d x)
    lt[:32, 88:96] = 1.0                    # pattern 11: block-0 ones (g=2 sx;
                                            # tap 8 is replicated in 4 blocks)
    sw = np.zeros((128, 1), np.float32)
    s_w = W_col.sum(axis=0)                 # [32]
    for c in range(COUT):
        sw[32 * (c // 8) + (c % 8), 0] = s_w[c]
    return wp, lt.astype(_BF16), sw


# ------------------------------------------------------------- device kernel
def _build_nc():
    from contextlib import ExitStack
    import concourse.tile as tile
    from concourse import bacc, mybir

    bf16, f32 = mybir.dt.bfloat16, mybir.dt.float32
    Alu = mybir.AluOpType
    Act = mybir.ActivationFunctionType

    sx_passes, passes = _build_passes()
    nc = bacc.Bacc("TRN2", target_bir_lowering=False, debug=False)
    xn_d = [nc.declare_dram_parameter(f"xn{g}", [128, F], bf16, False)
            for g in range(3)]
    wp_d = nc.declare_dram_parameter("wp", [128, 72], f32, False)
    lt_d = nc.declare_dram_parameter("lt", [128, 96], bf16, False)
    sw_d = nc.declare_dram_parameter("sw", [128, 1], f32, False)
    o_d = nc.declare_dram_parameter("o", [4, 8, F], bf16, True)

    with tile.TileContext(nc) as tc, ExitStack() as ctx:
        singles = ctx.enter_context(tc.tile_pool(name="singles", bufs=1))
        bvpool = ctx.enter_context(tc.tile_pool(name="bvpool", bufs=8))
        bapool = ctx.enter_context(tc.tile_pool(name="bapool", bufs=4))
        ppool = ctx.enter_context(tc.tile_pool(name="ppool", bufs=1, space="PSUM"))

        wp = singles.tile([128, 72], f32, tag="wp")
        lt = singles.tile([128, 96], bf16, tag="lt")
        sw = singles.tile([128, 1], f32, tag="sw")
        nc.scalar.dma_start(lt[:], lt_d[:])
        nc.scalar.dma_start(wp[:], wp_d[:])
        nc.scalar.dma_start(sw[:], sw_d[:])
        xn = []
        for g in range(3):
            t = singles.tile([128, F], bf16, tag=f"xn{g}")
            xn.append(t)
        nc.sync.dma_start(xn[0][:], xn_d[0][:])
        nc.gpsimd.dma_start(xn[1][:], xn_d[1][:])
        nc.scalar.dma_start(xn[2][:], xn_d[2][:])
        ost = singles.tile([128, F], bf16, tag="ost")
        P = ppool.tile([128, F], f32, tag="P")

        # Touchers: pre-sync each compute engine on the input DMAs so most
        # compute ops need a single semaphore wait.
        tv = singles.tile([128, 8], f32, tag="tv")
        ta = singles.tile([128, 8], f32, tag="ta")
        tg = singles.tile([128, 8], f32, tag="tg")
        nc.vector.tensor_copy(tv[:, 0:2], wp[:, 0:2])
        nc.scalar.copy(ta[:, 0:2], wp[:, 0:2])
        nc.gpsimd.tensor_copy(tg[:, 0:2], wp[:, 0:2])
        for g in range(3):
            nc.vector.tensor_copy(tv[:, 2 * g + 2:2 * g + 4], xn[g][:, 0:2])
            nc.scalar.copy(ta[:, 2 * g + 2:2 * g + 4], xn[g][:, 0:2])
            nc.gpsimd.tensor_copy(tg[:, 2 * g + 2:2 * g + 4], xn[g][:, 0:2])
        nc.tensor.ldweights(lt[:, 80:88])

        def emit_mms(rhs_tile, p):
            j = p["stripe"]
            lt_ap = lt[:, 8 * p["pat"]:8 * p["pat"] + 8]
            for (off, sz) in CHUNKS:
                nc.tensor.matmul(
                    P[32 * j:32 * j + 8, off:off + sz],
                    lt_ap, rhs_tile[:, off:off + sz],
                    start=p["start"], stop=p.get("stop", False),
                    tile_position=(0, 32 * j),
                )

        for p in sx_passes:
            emit_mms(xn[p["g"]], p)

        for i, p in enumerate(passes):
            scol = wp[:, i:i + 1]
            src = xn[p["g"]]
            if p["engine"] == "v":
                B = bvpool.tile([128, F], bf16, tag="BV")
                nc.vector.tensor_scalar(B[:], src[:], scol, 0.0,
                                        op0=Alu.add, op1=Alu.max)
            elif p["engine"] == "g":
                B = bapool.tile([128, F], bf16, tag="BG")
                nc.gpsimd.tensor_scalar(B[:], src[:], scol, 0.0,
                                        op0=Alu.add, op1=Alu.max)
            else:
                B = bapool.tile([128, F], bf16, tag="BA")
                nc.scalar.activation(B[:], src[:], Act.Relu, bias=scol)
            emit_mms(B, p)

        # epilogue: out = psum + S_w[c] (bias per partition).  One whole-psum
        # copy: engine cost scales with free size only, and a single
        # instruction avoids PSUM bank-overlap serialization.
        nc.vector.tensor_scalar(ost[:], P[:], sw[:], None, op0=Alu.add)
        for j in range(4):
            nc.sync.dma_start(o_d[j], ost[32 * j:32 * j + 8, :])
    nc.finalize()
    return nc


_NC_CACHE = None


def _get_nc():
    global _NC_CACHE
    if _NC_CACHE is None:
        _NC_CACHE = _build_nc()
    return _NC_CACHE


# -------------------------------------------------------------------- driver
def _run(x, kf, trace=False):
    from concourse.bass_utils import run_bass_kernel_spmd

    x = np.ascontiguousarray(np.asarray(x, np.float32))
    kf = np.ascontiguousarray(np.asarray(kf, np.float32))
    wp, lt, sw = _host_prep_weights(kf)
    in_maps = []
    for core in range(N_CORES):
        XN = _host_prep_core(x, core)
        in_maps.append({
            "xn0": XN[0].astype(_BF16),
            "xn1": XN[1].astype(_BF16),
            "xn2": XN[2].astype(_BF16),
            "wp": wp,
            "lt": lt,
            "sw": sw,
        })
    nc = _get_nc()
    res = run_bass_kernel_spmd(nc, in_maps, core_ids=list(range(N_CORES)),
                               trace=trace)
    out = np.zeros((4, H, W, COUT), np.float32)
    for core in range(N_CORES):
        o = np.asarray(res.results[core]["o"]).astype(np.float32)  # [4, 8, F]
        n, h0 = core // 2, (core % 2) * 32
        oo = o.reshape(4, 8, ROWS, WP)[:, :, :, 1:65]   # [4, 8, 32, 64]
        out[n, h0:h0 + 32] = oo.transpose(2, 3, 0, 1).reshape(ROWS, W, COUT)
    return out, res


def kernel(**inputs):
    out, _ = _run(inputs["x"], inputs["kernel"])
    return out
